# revision 5
# baseline (speedup 1.0000x reference)
"""OnlineTripletLoss (batch-hard mining) on 8 Trainium2 NeuronCores.

Strategy (anchors sharded across cores, per the sharding hint):
  * Host: sort anchors by label so each class is a contiguous column range,
    L2-normalize rows, transpose to eT [768, 4096] (f32).
  * Device, SPMD over 8 cores: core c computes its Gram slab
    G = eT[:, c*512:(c+1)*512].T @ eT  ([512, 4096]) with float32r matmuls,
    then per-class-segment min/max reductions along rows. Because
    dist = sqrt(relu(2 - 2*dot) + eps) is monotonically decreasing in dot,
    segment min/max of dot is sufficient: the distance transform is applied
    after reduction, on [4096, nclass] vectors, on the host.
  * Host: combine segment pieces, hardest-positive = f(min dot over own
    class), hardest-negative = f(max dot over other classes), hinge, mean.

Self-contained: hardcodes N=4096, D=768, 8 cores.
"""

import os

import numpy as np

N = 4096
D = 768
NCORES = 8
KCH = D // 128  # 6 contraction chunks of 128
SLAB = N // NCORES  # 512 anchors per core
MT = SLAB // 128  # 4 m-tiles per core
WIN = 1024  # columns per PSUM generation (8 banks = 4 m-tiles x 1024)
NGEN = N // WIN
MARGIN = 1.0
EPS = 1e-12

_CACHE = {}
LAST_RESULTS = None


def _build_program(bounds):
    """Build + compile the SPMD Bass program.

    bounds: tuple of (cls, lo, hi) sorted-column ranges, one per present class.
    Returns (nc, pieces, NP) where pieces[i] = (cls, lo, hi) are the
    window-clipped reduce segments and NP = len(pieces).
    """
    import concourse.tile as tile
    from concourse import bacc, mybir

    # Clip class segments to WIN-sized generation windows.
    pieces = []
    for cls, lo, hi in bounds:
        start = lo
        while start < hi:
            end = min(hi, (start // WIN + 1) * WIN)
            pieces.append((cls, start, end))
            start = end
    NP = len(pieces)
    piece_by_gen = {}
    for pidx, (cls, lo, hi) in enumerate(pieces):
        piece_by_gen.setdefault(lo // WIN, []).append((pidx, cls, lo, hi))

    nc = bacc.Bacc(
        "TRN2",
        target_bir_lowering=False,
        debug=False,
        enable_asserts=False,
        num_devices=NCORES,
    )
    f32 = mybir.dt.float32
    f32r = mybir.dt.float32r

    et = nc.dram_tensor("et", [KCH, 128, N], f32r, kind="ExternalInput").ap()
    slab = nc.dram_tensor("slab", [KCH, 128, SLAB], f32r, kind="ExternalInput").ap()
    out = nc.dram_tensor("out", [MT, 128, 2 * NP], f32, kind="ExternalOutput").ap()

    with tile.TileContext(nc) as tc:
        with (
            tc.tile_pool(name="singles", bufs=1) as singles,
            tc.tile_pool(name="y", bufs=4) as ypool,
            tc.tile_pool(name="ps", bufs=8, space="PSUM") as pspool,
        ):
            # lhsT slab first: every generation needs it.
            slab_sb = singles.tile([128, KCH, SLAB], f32r)
            for k in range(KCH):
                nc.sync.dma_start(out=slab_sb[:, k, :], in_=slab[k])

            # rhs columns arrive generation-by-generation so gen0 matmuls
            # start after ~1/4 of the full load.
            et_sb = [
                singles.tile([128, KCH, WIN], f32r, name=f"et{g}", tag=f"et{g}") for g in range(NGEN)
            ]
            pieces_sb = singles.tile([128, MT, 2 * NP], f32)
            for g in range(NGEN):
                for k in range(KCH):
                    nc.sync.dma_start(
                        out=et_sb[g][:, k, :], in_=et[k, :, g * WIN : (g + 1) * WIN]
                    )

            for g in range(NGEN):
                pt = {}
                for m in range(MT):
                    for j in range(2):
                        pt[(m, j)] = pspool.tile([128, 512], f32, name=f"ps_g{g}_m{m}_{j}", tag="ps")
                for k in range(KCH):
                    for m in range(MT):
                        lhsT = slab_sb[:, k, m * 128 : (m + 1) * 128]
                        for j in range(2):
                            rhs = et_sb[g][:, k, j * 512 : (j + 1) * 512]
                            nc.tensor.matmul(
                                pt[(m, j)],
                                lhsT,
                                rhs,
                                start=(k == 0),
                                stop=(k == KCH - 1),
                            )
                for m in range(MT):
                    y = ypool.tile([128, WIN], f32, name=f"y_g{g}_m{m}", tag="y")
                    for j in range(2):
                        nc.scalar.copy(y[:, j * 512 : (j + 1) * 512], pt[(m, j)])
                    for pidx, cls, lo, hi in piece_by_gen.get(g, ()):
                        seg = y[:, lo - g * WIN : hi - g * WIN]
                        nc.vector.tensor_reduce(
                            out=pieces_sb[:, m, pidx : pidx + 1],
                            in_=seg,
                            axis=mybir.AxisListType.X,
                            op=mybir.AluOpType.min,
                        )
                        nc.vector.tensor_reduce(
                            out=pieces_sb[:, m, NP + pidx : NP + pidx + 1],
                            in_=seg,
                            axis=mybir.AxisListType.X,
                            op=mybir.AluOpType.max,
                        )
            for m in range(MT):
                nc.sync.dma_start(out=out[m], in_=pieces_sb[:, m, :])

    nc.compile()
    return nc, pieces, NP


def kernel(embeddings, labels):
    global LAST_RESULTS
    from concourse.bass_utils import run_bass_kernel_spmd

    emb = np.asarray(embeddings, dtype=np.float32)
    lab = np.asarray(labels)
    assert emb.shape == (N, D)

    order = np.argsort(lab, kind="stable")
    lab_s = np.asarray(lab[order], dtype=np.int64)

    bounds = []
    i = 0
    while i < N:
        j = i
        while j < N and lab_s[j] == lab_s[i]:
            j += 1
        bounds.append((int(lab_s[i]), i, j))
        i = j
    key = tuple(bounds)
    if key not in _CACHE:
        _CACHE[key] = _build_program(key)
    nc, pieces, NP = _CACHE[key]

    nrm = np.sqrt((emb * emb).sum(axis=1))
    e = emb / np.maximum(nrm, EPS)[:, None]
    eT = np.ascontiguousarray(e[order].T)  # [768, 4096]
    et_in = np.ascontiguousarray(eT.reshape(KCH, 128, N))
    in_maps = []
    for c in range(NCORES):
        slab_in = np.ascontiguousarray(et_in[:, :, c * SLAB : (c + 1) * SLAB])
        in_maps.append({"et": et_in, "slab": slab_in})

    trace = bool(os.environ.get("KERNEL_TRACE"))
    res = run_bass_kernel_spmd(
        nc, in_maps, core_ids=list(range(NCORES)), trace=trace
    )
    LAST_RESULTS = res

    parts = np.concatenate(
        [res.results[c]["out"].reshape(MT * 128, 2 * NP) for c in range(NCORES)],
        axis=0,
    )  # [4096, 2*NP] in sorted-anchor order
    pmin, pmax = parts[:, :NP], parts[:, NP:]

    classes = [b[0] for b in bounds]
    col_of = {cls: [i for i, (c2, _, _) in enumerate(pieces) if c2 == cls] for cls in classes}
    Gmin = np.stack([pmin[:, col_of[c]].min(axis=1) for c in classes], axis=1)
    Gmax = np.stack([pmax[:, col_of[c]].max(axis=1) for c in classes], axis=1)

    # class index of each sorted row
    starts = [lo for (_, lo, _) in bounds]
    cls_idx = np.searchsorted(starts, np.arange(N), side="right") - 1
    rows = np.arange(N)

    own_min = Gmin[rows, cls_idx]  # smallest dot among own class = hardest positive
    d_ap = np.sqrt(np.maximum(2.0 - 2.0 * own_min, 0.0) + EPS)

    Gmax_neg = Gmax.copy()
    Gmax_neg[rows, cls_idx] = -np.inf  # exclude own class
    gneg = Gmax_neg.max(axis=1)  # largest dot among negatives = hardest negative
    with np.errstate(over="ignore"):
        d_an = np.sqrt(np.maximum(2.0 - 2.0 * gneg, 0.0) + EPS)

    counts = np.array([hi - lo for (_, lo, hi) in bounds])
    valid = counts[cls_idx] >= 2
    per = np.maximum(d_ap - d_an + MARGIN, 0.0)  # d_an=inf (no negatives) -> 0
    per = np.where(valid, per, 0.0)
    n_valid = max(float(valid.sum()), 1.0)
    return np.float32(per.sum() / n_valid)


# revision 7
# speedup vs baseline: 1.0352x; 1.0352x over previous
"""OnlineTripletLoss (batch-hard mining) on 8 Trainium2 NeuronCores.

Strategy (anchors sharded across cores, per the sharding hint):
  * Host: sort anchors by label so each class is a contiguous column range,
    L2-normalize rows, transpose to eT [768, 4096] (f32).
  * Device, SPMD over 8 cores: core c computes its Gram slab
    G = eT[:, c*512:(c+1)*512].T @ eT  ([512, 4096]) with float32r matmuls,
    then per-class-segment min/max reductions along rows. Because
    dist = sqrt(relu(2 - 2*dot) + eps) is monotonically decreasing in dot,
    segment min/max of dot is sufficient: the distance transform is applied
    after reduction, on [4096, nclass] vectors, on the host.
  * Host: combine segment pieces, hardest-positive = f(min dot over own
    class), hardest-negative = f(max dot over other classes), hinge, mean.

Self-contained: hardcodes N=4096, D=768, 8 cores.
"""

import os

import numpy as np

N = 4096
D = 768
NCORES = 8
KCH = D // 128  # 6 contraction chunks of 128
SLAB = N // NCORES  # 512 anchors per core
MT = SLAB // 128  # 4 m-tiles per core
WIN = 1024  # columns per PSUM generation (8 banks = 4 m-tiles x 1024)
NGEN = N // WIN
MARGIN = 1.0
EPS = 1e-12

_CACHE = {}
LAST_RESULTS = None


def _build_program(bounds):
    """Build + compile the SPMD Bass program.

    bounds: tuple of (cls, lo, hi) sorted-column ranges, one per present class.
    Returns (nc, pieces, NP) where pieces[i] = (cls, lo, hi) are the
    window-clipped reduce segments and NP = len(pieces).
    """
    import concourse.tile as tile
    from concourse import bacc, mybir

    # Generation windows: small first window so the PE starts after a short
    # DMA lead-in; small last window so the epilogue tail is short.
    windows = [512, 1024, 1024, 1024, 512]
    assert sum(windows) == N
    wstart = [0]
    for w in windows:
        wstart.append(wstart[-1] + w)

    # Clip class segments to generation windows.
    pieces = []
    piece_by_gen = {}
    for g in range(len(windows)):
        w0, w1 = wstart[g], wstart[g + 1]
        for cls, lo, hi in bounds:
            s, e = max(lo, w0), min(hi, w1)
            if s < e:
                piece_by_gen.setdefault(g, []).append((len(pieces), cls, s, e))
                pieces.append((cls, s, e))
    NP = len(pieces)

    nc = bacc.Bacc(
        "TRN2",
        target_bir_lowering=False,
        debug=False,
        enable_asserts=False,
        num_devices=NCORES,
    )
    f32 = mybir.dt.float32
    f32r = mybir.dt.float32r

    et = nc.dram_tensor("et", [KCH, 128, N], f32r, kind="ExternalInput").ap()
    slab = nc.dram_tensor("slab", [KCH, 128, SLAB], f32r, kind="ExternalInput").ap()
    out = nc.dram_tensor("out", [MT, 128, 2 * NP], f32, kind="ExternalOutput").ap()
    warm_out = nc.dram_tensor("warm_out", [128, 1], f32, kind="ExternalOutput").ap()

    NWARM = 26  # PE warmup matmuls covering the DMA lead-in (keeps HAM at 2.4GHz)

    with tile.TileContext(nc) as tc:
        with (
            tc.tile_pool(name="singles", bufs=1) as singles,
            tc.tile_pool(name="y", bufs=4) as ypool,
            tc.tile_pool(name="ps", bufs=8, space="PSUM") as pspool,
        ):
            # lhsT slab first: every generation needs it.
            slab_sb = singles.tile([128, KCH, SLAB], f32r)
            for k in range(KCH):
                nc.sync.dma_start(out=slab_sb[:, k, :], in_=slab[k])

            # Warmup: dummy accumulating matmuls on the first slab chunk keep
            # the PE busy (and un-throttled) while the rhs columns stream in.
            warm_ps = pspool.tile([128, 512], f32, name="warm_ps", tag="ps")
            for i in range(NWARM):
                nc.tensor.matmul(
                    warm_ps,
                    slab_sb[:, 0, 0:128],
                    slab_sb[:, 0, :],
                    start=(i == 0),
                    stop=(i == NWARM - 1),
                )
            warm_sb = singles.tile([128, 1], f32, name="warm_sb")
            nc.scalar.copy(warm_sb, warm_ps[:, 0:1])

            # rhs columns arrive window-by-window so gen0 matmuls start after
            # a short DMA lead-in.
            et_sb = [
                singles.tile([128, KCH, windows[g]], f32r, name=f"et{g}", tag=f"et{g}")
                for g in range(len(windows))
            ]
            pieces_sb = singles.tile([128, MT, 2 * NP], f32)
            for g in range(len(windows)):
                for k in range(KCH):
                    nc.sync.dma_start(
                        out=et_sb[g][:, k, :],
                        in_=et[k, :, wstart[g] : wstart[g + 1]],
                    )

            for g in range(len(windows)):
                nj = windows[g] // 512  # 512-column PSUM banks this window
                pt = {}
                for m in range(MT):
                    for j in range(nj):
                        pt[(m, j)] = pspool.tile(
                            [128, 512], f32, name=f"ps_g{g}_m{m}_{j}", tag="ps"
                        )
                for k in range(KCH):
                    for m in range(MT):
                        lhsT = slab_sb[:, k, m * 128 : (m + 1) * 128]
                        for j in range(nj):
                            rhs = et_sb[g][:, k, j * 512 : (j + 1) * 512]
                            nc.tensor.matmul(
                                pt[(m, j)],
                                lhsT,
                                rhs,
                                start=(k == 0),
                                stop=(k == KCH - 1),
                            )
                for m in range(MT):
                    y = ypool.tile(
                        [128, windows[g]], f32, name=f"y_g{g}_m{m}", tag="y"
                    )
                    for j in range(nj):
                        nc.scalar.copy(y[:, j * 512 : (j + 1) * 512], pt[(m, j)])
                    for pidx, cls, lo, hi in piece_by_gen.get(g, ()):
                        w = hi - lo
                        ev = w - (w % 2)  # even width -> DVE 2x mode
                        pmin = pieces_sb[:, m, pidx : pidx + 1]
                        pmax = pieces_sb[:, m, NP + pidx : NP + pidx + 1]
                        if ev:
                            seg = y[:, lo - wstart[g] : lo - wstart[g] + ev]
                            nc.vector.tensor_reduce(
                                out=pmin, in_=seg,
                                axis=mybir.AxisListType.X, op=mybir.AluOpType.min,
                            )
                            nc.vector.tensor_reduce(
                                out=pmax, in_=seg,
                                axis=mybir.AxisListType.X, op=mybir.AluOpType.max,
                            )
                        if w % 2:
                            last = y[:, hi - 1 - wstart[g] : hi - wstart[g]]
                            if ev:
                                nc.vector.tensor_tensor(pmin, pmin, last, mybir.AluOpType.min)
                                nc.vector.tensor_tensor(pmax, pmax, last, mybir.AluOpType.max)
                            else:
                                nc.vector.tensor_copy(pmin, last)
                                nc.vector.tensor_copy(pmax, last)
            for m in range(MT):
                nc.sync.dma_start(out=out[m], in_=pieces_sb[:, m, :])
            nc.sync.dma_start(out=warm_out, in_=warm_sb)

    nc.compile()
    return nc, pieces, NP


def kernel(embeddings, labels):
    global LAST_RESULTS
    from concourse.bass_utils import run_bass_kernel_spmd

    emb = np.asarray(embeddings, dtype=np.float32)
    lab = np.asarray(labels)
    assert emb.shape == (N, D)

    order = np.argsort(lab, kind="stable")
    lab_s = np.asarray(lab[order], dtype=np.int64)

    bounds = []
    i = 0
    while i < N:
        j = i
        while j < N and lab_s[j] == lab_s[i]:
            j += 1
        bounds.append((int(lab_s[i]), i, j))
        i = j
    key = tuple(bounds)
    if key not in _CACHE:
        _CACHE[key] = _build_program(key)
    nc, pieces, NP = _CACHE[key]

    nrm = np.sqrt((emb * emb).sum(axis=1))
    e = emb / np.maximum(nrm, EPS)[:, None]
    eT = np.ascontiguousarray(e[order].T)  # [768, 4096]
    et_in = np.ascontiguousarray(eT.reshape(KCH, 128, N))
    in_maps = []
    for c in range(NCORES):
        slab_in = np.ascontiguousarray(et_in[:, :, c * SLAB : (c + 1) * SLAB])
        in_maps.append({"et": et_in, "slab": slab_in})

    trace = bool(os.environ.get("KERNEL_TRACE"))
    res = run_bass_kernel_spmd(
        nc, in_maps, core_ids=list(range(NCORES)), trace=trace
    )
    LAST_RESULTS = res

    parts = np.concatenate(
        [res.results[c]["out"].reshape(MT * 128, 2 * NP) for c in range(NCORES)],
        axis=0,
    )  # [4096, 2*NP] in sorted-anchor order
    pmin, pmax = parts[:, :NP], parts[:, NP:]

    classes = [b[0] for b in bounds]
    col_of = {cls: [i for i, (c2, _, _) in enumerate(pieces) if c2 == cls] for cls in classes}
    Gmin = np.stack([pmin[:, col_of[c]].min(axis=1) for c in classes], axis=1)
    Gmax = np.stack([pmax[:, col_of[c]].max(axis=1) for c in classes], axis=1)

    # class index of each sorted row
    starts = [lo for (_, lo, _) in bounds]
    cls_idx = np.searchsorted(starts, np.arange(N), side="right") - 1
    rows = np.arange(N)

    own_min = Gmin[rows, cls_idx]  # smallest dot among own class = hardest positive
    d_ap = np.sqrt(np.maximum(2.0 - 2.0 * own_min, 0.0) + EPS)

    Gmax_neg = Gmax.copy()
    Gmax_neg[rows, cls_idx] = -np.inf  # exclude own class
    gneg = Gmax_neg.max(axis=1)  # largest dot among negatives = hardest negative
    with np.errstate(over="ignore"):
        d_an = np.sqrt(np.maximum(2.0 - 2.0 * gneg, 0.0) + EPS)

    counts = np.array([hi - lo for (_, lo, hi) in bounds])
    valid = counts[cls_idx] >= 2
    per = np.maximum(d_ap - d_an + MARGIN, 0.0)  # d_an=inf (no negatives) -> 0
    per = np.where(valid, per, 0.0)
    n_valid = max(float(valid.sum()), 1.0)
    return np.float32(per.sum() / n_valid)
